# revision 15
# baseline (speedup 1.0000x reference)
"""Trainium2 Bass kernel for the CGF tree-GRU problem.

Problem: 3-level complete 8-ary tree GRU (torch GRU cell convention).
  Level 3: 64 nodes x 8 embedded leaf children, h0 = 0
  Level 2:  8 nodes x 8 children (level-3 outputs), h0 = mean of children h
  Level 1:  1 node  x 8 children (level-2 outputs), h0 = mean of children h
  Output: mean over the 8 step outputs of the root GRU. D = 512.

The computation is ONE serial chain of 24 GRU steps; each step is bounded
below by moving W_hh through the PE array (LDWEIGHTS) plus a serial
DVE/ACT gate chain.  The kernel is replicated on all 8 cores (SPMD,
identical inputs); core 0's output is returned.  Sharding saves nothing:
the step cost is independent of the node-batch size and a per-step
collective costs more than a step.

Layout: transposed - gate/hidden dims on the 128 partitions, batch on the
free dim.  This makes biases per-partition ACT scalars and removes all
transposes.

Precision (validated vs the jax reference in fp emulation, 4.6e-3 final):
  - r,z recurrent matmuls + the level-3 input matmul run in fp8e4m3
    DoubleRow perf mode (2 contraction rows per PE pass -> half the
    LDWEIGHTS).  Weights are pre-scaled x64 and activations x16 to clear
    the fp8 subnormal range; PSUM therefore holds 1024x the torch values
    and the ACT de-scales for free via its input-scale argument.
  - the n-gate path (the additive, error-sensitive one) and the level-2/1
    input matmuls stay bf16 (x64/x16-scaled as well so PSUM domains match).
  - state h is kept twice: bf16 (x16) feeding the n matmuls and fp8 (x16)
    feeding the r,z DoubleRow matmuls.
Gate order per step is r -> n -> z so the sigmoid(r) and the n-combine
overlap the remaining bursts, and the blend is fused with
scalar_tensor_tensor: u = 16n - h; ft = z*u; h' = 16n - ft.
"""

import numpy as np

import concourse.bacc as bacc
import concourse.mybir as mybir
from concourse.tile import TileContext
from concourse.bass_utils import run_bass_kernel_spmd

AF = mybir.ActivationFunctionType
OP = mybir.AluOpType
PM = mybir.MatmulPerfMode.DoubleRow
FP = mybir.dt.float32
BF = mybir.dt.bfloat16
F8 = mybir.dt.float8e4

P = 128          # partitions
D = 512          # hidden size
KT = D // P      # 4 k-tiles (contraction)
MT = 12          # gate m-tiles (3*512/128)
A = 8            # tree arity == sequence length per level
NB = 64          # level-3 node count
T = 8            # steps per level
N_CORES = 8
WS = 64.0        # weight pre-scale (fp8 range)
HS = 16.0        # activation pre-scale
SS = WS * HS     # PSUM domain scale (1024)

TNB = T * NB     # 512 level-3 sequence columns

# fp8 blob: [xt(2048) | wit8(6144) | whrz8(4096)]
O_XT = 0
O_WIT8 = O_XT + KT * TNB
O_WHRZ = O_WIT8 + MT * KT * P
B8_COLS = O_WHRZ + 8 * KT * P
# bf16 blob: [whn16(2048) | wit16(6144)]
O_WHN = 0
O_WIT16 = O_WHN + 4 * KT * P
B16_COLS = O_WIT16 + MT * KT * P
# fp32 blob: [gb1024(12) | bhnb1024(256)]
B32_COLS = MT + KT * NB

_BUILT = None  # cached Bass module
DEBUG = False


def _build_nc():
    nc = bacc.Bacc()

    blob8 = nc.declare_dram_parameter("blob8", [P, B8_COLS], F8, isOutput=False)
    blob16 = nc.declare_dram_parameter("blob16", [P, B16_COLS], BF, isOutput=False)
    blob32 = nc.declare_dram_parameter("blob32", [P, B32_COLS], FP, isOutput=False)
    outp = nc.declare_dram_parameter("out", [P, KT], FP, isOutput=True)
    if DEBUG:
        d_gi3 = nc.declare_dram_parameter("d_gi3", [P, TNB], FP, isOutput=True)
        d_gin = nc.declare_dram_parameter("d_gin", [P, 4 * TNB], FP, isOutput=True)
        d_h80 = nc.declare_dram_parameter("d_h80", [P, KT * NB], FP, isOutput=True)
        d_hb0 = nc.declare_dram_parameter("d_hb0", [P, KT * NB], FP, isOutput=True)
        d_hbF = nc.declare_dram_parameter("d_hbF", [P, KT * NB], FP, isOutput=True)
        d_acc3 = nc.declare_dram_parameter("d_acc3", [P, KT * NB], FP, isOutput=True)
        d_s1 = {}
        for nm in ("arz_r", "rt", "q", "ct", "nt", "u", "ft", "zt", "h8", "hb"):
            d_s1[nm] = nc.declare_dram_parameter(f"d_s1_{nm}", [P, KT * NB], FP, isOutput=True)

    with TileContext(nc) as tc:
        with (
            tc.tile_pool(name="const", bufs=1) as cpool,
            tc.tile_pool(name="state", bufs=1) as spool,
            tc.tile_pool(name="work", bufs=2) as wpool,
            tc.tile_pool(name="pg", bufs=4, space="PSUM") as gpool,
            tc.tile_pool(name="pr", bufs=1, space="PSUM") as prpool,
            tc.tile_pool(name="pn", bufs=1, space="PSUM") as pnpool,
            tc.tile_pool(name="pz", bufs=1, space="PSUM") as pzpool,
        ):
            # Warm the activation tables up front; lazy ACT_TABLE_LOADs
            # otherwise stall the first sigmoid/tanh by >1us each.
            warm = cpool.tile([P, 8], FP)
            nc.vector.memset(warm[:, :], 0.0)
            for fn in (AF.Identity, AF.Sigmoid, AF.Tanh):
                nc.scalar.activation(warm[:, :], warm[:, :], fn)

            # Chunked input DMA in consumption order: xt8+wit8 feed the
            # level-3 input matmul immediately; whrz8/whn16 arrive during
            # it; wit16 (level-2/1 input weights) is needed ~40us in.
            b32_sb = cpool.tile([P, B32_COLS], FP)
            nc.scalar.dma_start(out=b32_sb[:], in_=blob32[:, :])
            b8_sb = cpool.tile([P, B8_COLS], F8)
            for i, c0 in enumerate(range(0, B8_COLS, 2048)):
                c1 = min(c0 + 2048, B8_COLS)
                eng = nc.sync if i % 2 == 0 else nc.scalar
                eng.dma_start(out=b8_sb[:, c0:c1], in_=blob8[:, c0:c1])
            b16_sb = cpool.tile([P, B16_COLS], BF)
            for i, c0 in enumerate(range(0, B16_COLS, 1024)):
                c1 = min(c0 + 1024, B16_COLS)
                eng = nc.scalar if i % 2 == 0 else nc.sync
                eng.dma_start(out=b16_sb[:, c0:c1], in_=blob16[:, c0:c1])

            xt8 = b8_sb[:, O_XT : O_XT + KT * TNB]
            wit8 = b8_sb[:, O_WIT8 : O_WIT8 + MT * KT * P]
            whrz8 = b8_sb[:, O_WHRZ : O_WHRZ + 8 * KT * P]
            whn16 = b16_sb[:, O_WHN : O_WHN + 4 * KT * P]
            wit16 = b16_sb[:, O_WIT16 : O_WIT16 + MT * KT * P]
            gb_sb = b32_sb[:, 0:MT]
            bhnb_sb = b32_sb[:, MT : MT + KT * NB]
            bhnbv = bhnb_sb.rearrange("p (k b) -> p k b", k=KT)

            def w8_tile(base, m, kk):
                # [P, 2, 128] DoubleRow stationary pair (k-tiles 2kk, 2kk+1)
                return base[
                    :, (m * KT + 2 * kk) * P : (m * KT + 2 * kk + 2) * P
                ].rearrange("p (two f) -> p two f", two=2)

            def w16_tile(base, m, k):
                return base[:, (m * KT + k) * P : (m * KT + k + 1) * P]

            # ---------------- Level 3 input matmul (fp8 DoubleRow) --------
            xt8v = xt8.rearrange("p (k c) -> p k c", k=KT)
            gi3 = cpool.tile([P, MT * TNB], BF)
            for m in range(MT):
                psb = gpool.tile([P, 512], FP, tag="gi", name="psb")
                ps = psb[:, :TNB]
                for c in range(2):
                    sl = ps[:, c * 256 : (c + 1) * 256]
                    for kk in range(2):
                        nc.tensor.matmul(
                            sl,
                            lhsT=w8_tile(wit8, m, kk),
                            rhs=xt8v[:, 2 * kk : 2 * kk + 2, c * 256 : (c + 1) * 256],
                            start=(kk == 0),
                            stop=(kk == 1),
                            perf_mode=PM,
                        )
                dst = gi3[:, m * TNB : (m + 1) * TNB]
                # alternate engines so the copies drain two banks at a time
                if m % 2 == 0:
                    nc.vector.tensor_scalar_add(dst, ps[:, :], gb_sb[:, m : m + 1])
                else:
                    nc.scalar.activation(
                        dst, ps[:, :], AF.Identity, bias=gb_sb[:, m : m + 1], scale=1.0
                    )

            def gru_level(B, h8, hb, acc, gi_v, zero_h0, dbg=None):
                """8 GRU steps.  h8: [P, KT*B] fp8 (x16) state feeding the
                r,z DoubleRow matmuls; hb: bf16 (x16) state feeding the n
                matmuls; acc: fp32 output accumulator (x16 domain).
                gi_v: [P, m, t, b] AP of the 1024-domain input gates."""
                W4 = 4 * B

                def h8kk(kk):
                    return h8[:, 2 * kk * B : (2 * kk + 2) * B].rearrange(
                        "p (two b) -> p two b", two=2
                    )

                for t in range(T):
                    gi_r = gi_v[:, 0:4, t]
                    gi_z = gi_v[:, 4:8, t]
                    gi_n = gi_v[:, 8:12, t]
                    rt = wpool.tile([P, W4], BF, tag="rt")
                    zt = wpool.tile([P, W4], BF, tag="zt")
                    nt = wpool.tile([P, W4], BF, tag="nt")
                    ctm = wpool.tile([P, W4], BF, tag="ctm")
                    ct = wpool.tile([P, W4], BF, tag="ct")
                    ft = wpool.tile([P, W4], BF, tag="ft")

                    def v(ap):
                        return ap.rearrange("p (m b) -> p m b", m=4)

                    if t == 0 and zero_h0:
                        # h = 0: gates come straight from gi; h' = (1-z)*n
                        nc.scalar.activation(v(rt[:, :]), gi_r, AF.Sigmoid, scale=1.0 / SS)
                        nc.scalar.activation(v(zt[:, :]), gi_z, AF.Sigmoid, scale=1.0 / SS)
                        nc.vector.tensor_mul(v(ctm[:, :]), v(rt[:, :]), bhnbv[:, :, :B])
                        nc.vector.tensor_add(v(ct[:, :]), v(ctm[:, :]), gi_n)
                        nc.scalar.activation(nt[:, :], ct[:, :], AF.Tanh, scale=1.0 / SS)
                        # ft = 16*n*z ; h' = 16*n - ft = 16*(1-z)*n
                        nc.vector.scalar_tensor_tensor(
                            ft[:, :], nt[:, :], HS, zt[:, :], OP.mult, OP.mult
                        )
                        nc.vector.scalar_tensor_tensor(
                            h8[:, :], nt[:, :], HS, ft[:, :], OP.mult, OP.subtract
                        )
                        nc.vector.scalar_tensor_tensor(
                            hb[:, :], nt[:, :], HS, ft[:, :], OP.mult, OP.subtract
                        )
                        nc.gpsimd.tensor_copy(acc[:, :], hb[:, :])
                        if dbg is not None:
                            dbg(t, None)
                        continue

                    ps_r = prpool.tile([P, 512], FP, tag="ps_r")
                    ps_n = pnpool.tile([P, 512], FP, tag="ps_n")
                    ps_z = pzpool.tile([P, 512], FP, tag="ps_z")
                    # bursts: r (fp8 DR) -> n (bf16) -> z (fp8 DR)
                    for m in range(4):
                        for kk in range(2):
                            nc.tensor.matmul(
                                ps_r[:, m * B : (m + 1) * B],
                                lhsT=w8_tile(whrz8, m, kk),
                                rhs=h8kk(kk),
                                start=(kk == 0),
                                stop=(kk == 1),
                                perf_mode=PM,
                            )
                    for m in range(4):
                        for k in range(KT):
                            nc.tensor.matmul(
                                ps_n[:, m * B : (m + 1) * B],
                                lhsT=w16_tile(whn16, m, k),
                                rhs=hb[:, k * B : (k + 1) * B],
                                start=(k == 0),
                                stop=(k == KT - 1),
                            )
                    for m in range(4):
                        for kk in range(2):
                            nc.tensor.matmul(
                                ps_z[:, m * B : (m + 1) * B],
                                lhsT=w8_tile(whrz8, m + 4, kk),
                                rhs=h8kk(kk),
                                start=(kk == 0),
                                stop=(kk == 1),
                                perf_mode=PM,
                            )

                    arz_r = wpool.tile([P, W4], BF, tag="arz_r")
                    arz_z = wpool.tile([P, W4], BF, tag="arz_z")
                    q = wpool.tile([P, W4], BF, tag="q")
                    u = wpool.tile([P, W4], BF, tag="u")
                    # Emission must follow dataflow order: the tile framework
                    # binds each read to the last write emitted before it.
                    nc.vector.tensor_add(v(arz_r[:, :]), v(ps_r[:, :W4]), gi_r)
                    nc.scalar.activation(rt[:, :], arz_r[:, :], AF.Sigmoid, scale=1.0 / SS)
                    nc.vector.tensor_add(v(q[:, :]), v(ps_n[:, :W4]), bhnbv[:, :, :B])
                    nc.vector.tensor_mul(ctm[:, :], q[:, :], rt[:, :])
                    nc.vector.tensor_add(v(ct[:, :]), v(ctm[:, :]), gi_n)
                    nc.vector.tensor_add(v(arz_z[:, :]), v(ps_z[:, :W4]), gi_z)
                    nc.scalar.activation(nt[:, :], ct[:, :], AF.Tanh, scale=1.0 / SS)
                    nc.scalar.activation(zt[:, :], arz_z[:, :], AF.Sigmoid, scale=1.0 / SS)
                    # blend: u = 16n - h ; ft = z*u ; h' = 16n - ft
                    nc.vector.scalar_tensor_tensor(
                        u[:, :], nt[:, :], HS, hb[:, :], OP.mult, OP.subtract
                    )
                    nc.vector.tensor_mul(ft[:, :], zt[:, :], u[:, :])
                    nc.vector.scalar_tensor_tensor(
                        h8[:, :], nt[:, :], HS, ft[:, :], OP.mult, OP.subtract
                    )
                    # bf16 state + output accumulation off the critical path
                    # (Pool lacks scalar_tensor_tensor: 16n - ft == (u - ft) + hb)
                    tmp = wpool.tile([P, W4], BF, tag="tmp")
                    nc.gpsimd.tensor_sub(tmp[:, :], u[:, :], ft[:, :])
                    nc.gpsimd.tensor_add(hb[:, :], tmp[:, :], hb[:, :])
                    if t == 0:
                        nc.gpsimd.tensor_copy(acc[:, :], hb[:, :])
                    else:
                        nc.gpsimd.tensor_add(acc[:, :], acc[:, :], hb[:, :])
                    if dbg is not None:
                        dbg(t, dict(arz_r=arz_r, rt=rt, q=q, ct=ct, nt=nt,
                                    u=u, ft=ft, zt=zt, h8=h8, hb=hb))

            # ---------------- Level 3: 64 nodes ----------------
            gi3v = gi3[:].rearrange("p (m t b) -> p m t b", m=MT, t=T)
            h83 = spool.tile([P, KT * NB], F8)
            hb3 = spool.tile([P, KT * NB], BF)
            acc3 = spool.tile([P, KT * NB], FP)
            dbg3 = None
            if DEBUG:
                dcp = cpool.tile([P, TNB], FP)
                nc.scalar.copy(dcp[:, :], gi3[:, :TNB])
                nc.sync.dma_start(out=d_gi3[:, :], in_=dcp[:, :])
                dcn = cpool.tile([P, 4 * TNB], FP)
                nc.vector.tensor_copy(dcn[:, :], gi3[:, 8 * TNB : 12 * TNB])
                nc.sync.dma_start(out=d_gin[:, :], in_=dcn[:, :])

                def dbg3(t, tiles):
                    if t == 1:
                        for nm, tl in tiles.items():
                            cc = cpool.tile([P, KT * NB], FP, name="cc", tag=f"ds1{nm}")
                            nc.scalar.copy(cc[:, :], tl[:, :])
                            nc.sync.dma_start(out=d_s1[nm][:, :], in_=cc[:, :])
                    if t == 0:
                        c0 = cpool.tile([P, KT * NB], FP, name="c0", tag="dc0")
                        nc.scalar.copy(c0[:, :], h83[:, :])
                        nc.sync.dma_start(out=d_h80[:, :], in_=c0[:, :])
                        c1 = cpool.tile([P, KT * NB], FP, name="c1", tag="dc1")
                        nc.scalar.copy(c1[:, :], hb3[:, :])
                        nc.sync.dma_start(out=d_hb0[:, :], in_=c1[:, :])
                    if t == T - 1:
                        c2 = cpool.tile([P, KT * NB], FP, name="c2", tag="dc2")
                        nc.scalar.copy(c2[:, :], hb3[:, :])
                        nc.sync.dma_start(out=d_hbF[:, :], in_=c2[:, :])
                        nc.sync.dma_start(out=d_acc3[:, :], in_=acc3[:, :])
            gru_level(NB, h83, hb3, acc3, gi3v, zero_h0=True, dbg=dbg3)

            # ---------------- Level 3 -> 2 transition ----------------
            # x2 = acc3/8 reordered (k,j,t)->(k,t,j) so step-t gi slices are
            # contiguous; stays in the x16 bf16 domain.
            x2 = spool.tile([P, KT * NB], BF)
            nc.scalar.mul(
                x2[:].rearrange("p (k t j) -> p k t j", k=KT, t=A),
                acc3[:].rearrange("p (k j t) -> p k t j", k=KT, j=A),
                1.0 / A,
            )
            hr2 = spool.tile([P, KT * A], FP)
            nc.vector.tensor_reduce(
                hr2[:].rearrange("p (k j) -> p k j", k=KT),
                hb3[:].rearrange("p (k j c) -> p k j c", k=KT, j=A),
                axis=mybir.AxisListType.X,
                op=OP.add,
            )
            h2b = spool.tile([P, KT * A], BF)
            nc.scalar.mul(h2b[:, :], hr2[:, :], 1.0 / A)
            h28 = spool.tile([P, KT * A], F8)
            nc.scalar.mul(h28[:, :], hr2[:, :], 1.0 / A)

            # ---------------- Level 2 input matmul (bf16) ----------------
            gi2 = cpool.tile([P, MT * NB], BF)
            for m in range(MT):
                psb = gpool.tile([P, 512], FP, tag="gi", name="psb")
                ps = psb[:, :NB]
                for k in range(KT):
                    nc.tensor.matmul(
                        ps[:, :],
                        lhsT=w16_tile(wit16, m, k),
                        rhs=x2[:, k * NB : (k + 1) * NB],
                        start=(k == 0),
                        stop=(k == KT - 1),
                    )
                dst = gi2[:, m * NB : (m + 1) * NB]
                if m % 2 == 0:
                    nc.vector.tensor_scalar_add(dst, ps[:, :], gb_sb[:, m : m + 1])
                else:
                    nc.scalar.activation(
                        dst, ps[:, :], AF.Identity, bias=gb_sb[:, m : m + 1], scale=1.0
                    )

            gi2v = gi2[:].rearrange("p (m t b) -> p m t b", m=MT, t=T)
            acc2 = spool.tile([P, KT * A], FP)
            gru_level(A, h28, h2b, acc2, gi2v, zero_h0=False)

            # ---------------- Level 2 -> 1 transition ----------------
            x1 = spool.tile([P, KT * A], BF)
            nc.scalar.mul(x1[:, :], acc2[:, :], 1.0 / A)
            hr1 = spool.tile([P, KT], FP)
            nc.vector.tensor_reduce(
                hr1[:].rearrange("p (k o) -> p k o", k=KT),
                h2b[:].rearrange("p (k o j) -> p k o j", k=KT, o=1),
                axis=mybir.AxisListType.X,
                op=OP.add,
            )
            h1b = spool.tile([P, KT], BF)
            nc.scalar.mul(h1b[:, :], hr1[:, :], 1.0 / A)
            h18 = spool.tile([P, KT], F8)
            nc.scalar.mul(h18[:, :], hr1[:, :], 1.0 / A)

            # ---------------- Level 1 input matmul ----------------
            gi1 = cpool.tile([P, MT * A], BF)
            for m in range(MT):
                psb = gpool.tile([P, 512], FP, tag="gi", name="psb")
                ps = psb[:, :A]
                for k in range(KT):
                    nc.tensor.matmul(
                        ps[:, :],
                        lhsT=w16_tile(wit16, m, k),
                        rhs=x1[:, k * A : (k + 1) * A],
                        start=(k == 0),
                        stop=(k == KT - 1),
                    )
                dst = gi1[:, m * A : (m + 1) * A]
                if m % 2 == 0:
                    nc.vector.tensor_scalar_add(dst, ps[:, :], gb_sb[:, m : m + 1])
                else:
                    nc.scalar.activation(
                        dst, ps[:, :], AF.Identity, bias=gb_sb[:, m : m + 1], scale=1.0
                    )

            gi1v = gi1[:].rearrange("p (m t b) -> p m t b", m=MT, t=T, b=1)
            acc1 = spool.tile([P, KT], FP)
            gru_level(1, h18, h1b, acc1, gi1v, zero_h0=False)

            out_sb = spool.tile([P, KT], FP)
            nc.scalar.mul(out_sb[:, :], acc1[:, :], 1.0 / (HS * A))
            nc.sync.dma_start(out=outp[:, :], in_=out_sb[:, :])

    nc.finalize()
    return nc


def _get_nc():
    global _BUILT
    if _BUILT is None:
        _BUILT = _build_nc()
    return _BUILT


def make_inputs(leaf_ids, embed_table, W_ih, W_hh, b_ih, b_hh):
    """Host-side layout prep: gather embedding rows, pre-scale, pack the
    transposed tile formats, quantize."""
    import ml_dtypes

    E4 = ml_dtypes.float8_e4m3
    BFnp = ml_dtypes.bfloat16

    leaf_ids = np.asarray(leaf_ids).astype(np.int64)
    emb = np.asarray(embed_table, dtype=np.float32)
    W_ih = np.asarray(W_ih, dtype=np.float32)
    W_hh = np.asarray(W_hh, dtype=np.float32)
    b_ih = np.asarray(b_ih, dtype=np.float32)
    b_hh = np.asarray(b_hh, dtype=np.float32)

    x = emb[leaf_ids]  # [64, 8, 512]
    xtm = np.ascontiguousarray(x.transpose(1, 0, 2)).reshape(TNB, D)
    xt = np.ascontiguousarray(
        xtm.T.reshape(KT, P, TNB).transpose(1, 0, 2)
    ).reshape(P, KT * TNB) * HS

    def pack_w(Wsub, scale):  # [rows, 512] -> [(m,k)-major lhsT tiles]
        WT = np.ascontiguousarray(Wsub.T) * scale  # [512, rows]
        mt = Wsub.shape[0] // P
        return np.ascontiguousarray(
            WT.reshape(KT, P, mt, P).transpose(1, 2, 0, 3)
        ).reshape(P, mt * KT * P)

    wit = pack_w(W_ih, WS)
    blob8 = np.concatenate([xt, wit, pack_w(W_hh[: 2 * D], WS)], axis=1).astype(E4)
    blob16 = np.concatenate([pack_w(W_hh[2 * D :], WS), wit], axis=1).astype(BFnp)

    gbias = SS * np.concatenate([(b_ih + b_hh)[: 2 * D], b_ih[2 * D :]])
    gb_in = np.ascontiguousarray(gbias.reshape(MT, P).T)
    bhn_in = np.ascontiguousarray((SS * b_hh[2 * D :]).reshape(KT, P).T)
    bhnb_in = np.ascontiguousarray(np.repeat(bhn_in, NB, axis=1))
    blob32 = np.concatenate([gb_in, bhnb_in], axis=1)

    assert blob8.shape == (P, B8_COLS)
    assert blob16.shape == (P, B16_COLS)
    assert blob32.shape == (P, B32_COLS)
    return {
        "blob8": np.ascontiguousarray(blob8),
        "blob16": np.ascontiguousarray(blob16),
        "blob32": np.ascontiguousarray(blob32),
    }


def unpack_output(out_np):
    # out [P, KT]: element (p, k) = root dim k*128+p
    return np.ascontiguousarray(out_np.T).reshape(1, 1, D).astype(np.float32)


def kernel(leaf_ids=None, layer=None, embed_table=None, W_ih=None, W_hh=None,
           b_ih=None, b_hh=None, **_unused):
    in_map = make_inputs(leaf_ids, embed_table, W_ih, W_hh, b_ih, b_hh)
    nc = _get_nc()
    res = run_bass_kernel_spmd(nc, [in_map] * N_CORES, list(range(N_CORES)))
    return unpack_output(res.results[0]["out"])
